# revision 1
# baseline (speedup 1.0000x reference)
"""Trainium2 Bass kernel for nn_AVIN_6794638262657 (topk_masking).

Computes, for B=192, C=512, H=W=28:
  fa  = relu(ea @ Wa1.T) @ Wa2.T
  fv  = einsum('bchw,oc->bohw', ev, Wv);  ind_vec = fv.mean((2,3))
  S   = <l2norm_c(fv), l2norm_c(ind_vec)>  -> [B, B, HW]
  per-(b,d) top-k (k=23) sigmoid masks from both ends -> SP, SN -> two CE losses
  plus a pairwise-distance loss between ind_vec and fa.
Returns ((loss1+loss2)/2, (loss3+loss4)/2).

Sharding: data-parallel over B across 8 NeuronCores (24 rows each);
ind_vec / fa / SP^T / SN^T are all-gathered via device collectives.
"""
import numpy as np

import concourse.bacc as bacc
from concourse import mybir
from concourse.tile import TileContext
from concourse.bass_utils import run_bass_kernel_spmd

# problem constants (hardcoded per spec)
B, C, H, W = 192, 512, 28, 28
HW = H * W                     # 784
NCORE = 8
BL = B // NCORE                # 24 rows of b per core
NPAIR = BL // 2                # 12
K4 = C // 128                  # 4 partition chunks of the channel dim
TS = 0.03
TC = 0.07
EPS = 1e-6
TS2 = 2.0 * TS
NSL = [(0, 512), (512, HW - 512)]   # free-dim slices for matmuls (<=512)
BIGNEG = -1e30

F32 = mybir.dt.float32
F32R = mybir.dt.float32r
AF = mybir.ActivationFunctionType
ALU = mybir.AluOpType
AX = mybir.AxisListType


def _rearr_kp(ap, p=128):
    """[ (k p), n ] DRAM view -> [p, k, n]"""
    return ap.rearrange("(k p) n -> p k n", p=p)


def build(stage=3):
    nc = bacc.Bacc("TRN2", target_bir_lowering=False, debug=False,
                   num_devices=NCORE)

    # ---- external I/O ----
    ev = nc.declare_dram_parameter("ev", [BL, C, HW], F32R, isOutput=False)
    eaT = nc.declare_dram_parameter("eaT", [2048, BL], F32, isOutput=False)
    Wv = nc.declare_dram_parameter("Wv", [C, C], F32, isOutput=False)
    WvT = nc.declare_dram_parameter("WvT", [C, C], F32R, isOutput=False)
    Wa1T = nc.declare_dram_parameter("Wa1T", [2048, C], F32, isOutput=False)
    Wa2T = nc.declare_dram_parameter("Wa2T", [C, C], F32, isOutput=False)

    loss12 = nc.declare_dram_parameter("loss12", [1, 1], F32, isOutput=True)
    loss34 = nc.declare_dram_parameter("loss34", [1, 1], F32, isOutput=True)
    spt_out = nc.declare_dram_parameter("spt", [B, B], F32, isOutput=True)
    snt_out = nc.declare_dram_parameter("snt", [B, B], F32, isOutput=True)
    ivt_out = nc.declare_dram_parameter("ivt", [C, B], F32, isOutput=True)
    fat_out = nc.declare_dram_parameter("fat", [C, B], F32, isOutput=True)

    # ---- internal DRAM ----
    recip_dram = nc.dram_tensor("recip_dram", [BL, HW], F32)
    nrm2_dram = nc.dram_tensor("nrm2_dram", [BL, HW], F32)
    srow_dram = nc.dram_tensor("srow_dram", [1, B], F32)
    rrow_dram = nc.dram_tensor("rrow_dram", [1, B], F32)
    rtrow_dram = nc.dram_tensor("rtrow_dram", [1, B], F32)
    ag1_in = nc.dram_tensor("ag1_in", [2 * C, BL], F32)
    ag1_out = nc.dram_tensor("ag1_out", [NCORE, 2 * C, BL], F32,
                             addr_space="Shared")
    ag2_in = nc.dram_tensor("ag2_in", [2, B, BL], F32)
    ag2_out = nc.dram_tensor("ag2_out", [NCORE, 2, B, BL], F32,
                             addr_space="Shared")

    groups = [list(range(NCORE))]

    with TileContext(nc) as tc:
        from contextlib import ExitStack
        ctx = ExitStack()
        with ctx:
            persist = ctx.enter_context(tc.tile_pool(name="persist", bufs=1))
            # persistent weights
            WvT_sb = persist.tile([128, K4, C], F32R)
            Wv_sb = persist.tile([128, K4, C], F32)
            nc.sync.dma_start(out=WvT_sb, in_=_rearr_kp(WvT[:]))
            nc.sync.dma_start(out=Wv_sb, in_=_rearr_kp(Wv[:]))
            ones_f = persist.tile([128, 1], F32)
            nc.vector.memset(ones_f, 1.0)
            ones_r = persist.tile([128, 1], F32R)
            nc.scalar.copy(ones_r[:], ones_f[:])

            # identity matrix for PE transposes & diag extraction
            ident = persist.tile([128, 128], F32)
            iota_p = persist.tile([128, 1], mybir.dt.int32)
            nc.gpsimd.iota(iota_p, pattern=[[0, 1]], base=0, channel_multiplier=1)
            iota_pf = persist.tile([128, 1], F32)
            nc.scalar.copy(iota_pf, iota_p[:])
            iota_r = persist.tile([128, 128], mybir.dt.int32)
            nc.gpsimd.iota(iota_r, pattern=[[1, 128]], base=0, channel_multiplier=0)
            iota_rf = persist.tile([128, 128], F32)
            nc.scalar.copy(iota_rf, iota_r[:])
            nc.vector.tensor_scalar(ident[:], iota_rf[:], iota_pf[:], None,
                                    op0=ALU.is_equal)

            # persistent result/state tiles
            ivT_sb = persist.tile([128, K4, BL], F32)      # local ind_vec^T
            faT_sb = persist.tile([128, K4, BL], F32)      # local fa^T
            nrm2_all = persist.tile([BL, HW], F32)
            recip_all = persist.tile([BL, HW], F32)
            ivT_full = persist.tile([128, K4, B], F32)
            faT_full = persist.tile([128, K4, B], F32)
            ivnT_sb = persist.tile([128, K4, B], F32)
            ivsq = persist.tile([128, K4, B], F32)
            UT_sb = persist.tile([128, K4, B], F32R)
            # per-row batched accumulators for the topk/mask phase
            NCOLS = BL + NPAIR  # 36
            T1S_arr = persist.tile([128, NCOLS], F32)  # sum tanh((x-t)/2TS)
            T1Z_arr = persist.tile([128, NCOLS], F32)
            LS_arr = persist.tile([128, NCOLS], F32)   # sum silu((x-t)/TS)
            LZ_arr = persist.tile([128, NCOLS], F32)
            BS_arr = persist.tile([128, NCOLS], F32)   # tanh biases = -t/(2TS)
            BZ_arr = persist.tile([128, NCOLS], F32)
            BS2_arr = persist.tile([128, NCOLS], F32)  # silu biases = -t/TS
            BZ2_arr = persist.tile([128, NCOLS], F32)
            SP_arr = persist.tile([128, NCOLS], F32)
            SN_arr = persist.tile([128, NCOLS], F32)
            zeros_hw = persist.tile([128, HW], F32)
            nc.gpsimd.memset(zeros_hw, 0.0)

            # ---------------- phase 0: audio path ----------------
            with tc.tile_pool(name="audio", bufs=1) as apool, \
                 tc.tile_pool(name="audio_ps", bufs=2, space="PSUM") as apsum:
                Wa1T_sb = apool.tile([128, 16, C], F32)
                nc.sync.dma_start(out=Wa1T_sb, in_=_rearr_kp(Wa1T[:]))
                Wa2T_sb = apool.tile([128, K4, C], F32)
                nc.sync.dma_start(out=Wa2T_sb, in_=_rearr_kp(Wa2T[:]))
                eaT_sb = apool.tile([128, 16, BL], F32)
                nc.sync.dma_start(out=eaT_sb, in_=_rearr_kp(eaT[:]))
                hT_sb = apool.tile([128, K4, BL], F32)
                for m in range(K4):
                    ph = apsum.tile([128, BL], F32)
                    for k in range(16):
                        nc.tensor.matmul(
                            out=ph[:], lhsT=Wa1T_sb[:, k, m * 128:(m + 1) * 128],
                            rhs=eaT_sb[:, k, :], start=(k == 0), stop=(k == 15))
                    nc.scalar.activation(hT_sb[:, m, :], ph[:], AF.Relu)
                for m in range(K4):
                    pf = apsum.tile([128, BL], F32)
                    for k in range(K4):
                        nc.tensor.matmul(
                            out=pf[:], lhsT=Wa2T_sb[:, k, m * 128:(m + 1) * 128],
                            rhs=hT_sb[:, k, :], start=(k == 0), stop=(k == K4 - 1))
                    nc.scalar.copy(faT_sb[:, m, :], pf[:])

            # ---------------- phase 1: y / nrm2 / evmean ----------------
            evpool = ctx.enter_context(tc.tile_pool(name="evp", bufs=2))
            with tc.tile_pool(name="p1sq", bufs=5) as sqpool, \
                 tc.tile_pool(name="p1n2row", bufs=2) as n2rowpool, \
                 tc.tile_pool(name="p1ps", bufs=2, space="PSUM") as ypsum, \
                 tc.tile_pool(name="p1n2", bufs=2, space="PSUM") as n2psum:
                evmT = persist.tile([128, K4, BL], F32)
                for b in range(BL):
                    ev_sb = evpool.tile([128, K4, HW], F32R, tag="ev")
                    nc.sync.dma_start(out=ev_sb, in_=_rearr_kp(ev[b]))
                    ysqs = []
                    for m in range(K4):
                        py = ypsum.tile([128, HW], F32, tag="y")
                        for (n0, nw) in NSL:
                            for k in range(K4):
                                nc.tensor.matmul(
                                    out=py[:, n0:n0 + nw],
                                    lhsT=WvT_sb[:, k, m * 128:(m + 1) * 128],
                                    rhs=ev_sb[:, k, n0:n0 + nw],
                                    start=(k == 0), stop=(k == K4 - 1))
                        ysq = sqpool.tile([128, HW], F32R, tag="ysq")
                        nc.scalar.square(ysq[:], py[:])
                        ysqs.append(ysq)
                    pn2 = n2psum.tile([1, HW], F32, tag="n2")
                    for m in range(K4):
                        for (n0, nw) in NSL:
                            nc.tensor.matmul(
                                out=pn2[0:1, n0:n0 + nw], lhsT=ones_r[:],
                                rhs=ysqs[m][:, n0:n0 + nw],
                                start=(m == 0), stop=(m == K4 - 1),
                                skip_group_check=True)
                    n2row = n2rowpool.tile([1, HW], F32, tag="n2row")
                    nc.scalar.copy(n2row[:], pn2[0:1, :])
                    nc.sync.dma_start(out=nrm2_dram[b:b + 1, :], in_=n2row[:])
                    for k in range(K4):
                        nc.vector.tensor_reduce(
                            out=evmT[:, k, b:b + 1],
                            in_=ev_sb[:, k, :].bitcast(F32), axis=AX.X, op=ALU.add)

            # recip_all = 1/sqrt(nrm2), with one Newton step for accuracy
            with tc.tile_pool(name="rp", bufs=1) as rpool:
                nc.sync.dma_start(out=nrm2_all[:], in_=nrm2_dram[:])
                sq_t = rpool.tile([BL, HW], F32)
                nc.scalar.sqrt(sq_t[:], nrm2_all[:])
                r0 = rpool.tile([BL, HW], F32)
                nc.vector.reciprocal(r0[:], sq_t[:])
                t0 = rpool.tile([BL, HW], F32)
                nc.vector.tensor_mul(t0[:], r0[:], r0[:])
                nc.vector.tensor_mul(t0[:], t0[:], nrm2_all[:])
                nc.vector.tensor_scalar(t0[:], t0[:], -0.5, 1.5,
                                        op0=ALU.mult, op1=ALU.add)
                nc.vector.tensor_mul(recip_all[:], r0[:], t0[:])
                nc.sync.dma_start(out=recip_dram[:], in_=recip_all[:])

                # ind_vec^T = (Wv @ evmean)/HW  (fp32 exact)
                with tc.tile_pool(name="ivps", bufs=2, space="PSUM") as ivpsum:
                    for m in range(K4):
                        piv = ivpsum.tile([128, BL], F32)
                        for k in range(K4):
                            nc.tensor.matmul(
                                out=piv[:],
                                lhsT=WvT_sb[:, k, m * 128:(m + 1) * 128].bitcast(F32),
                                rhs=evmT[:, k, :], start=(k == 0),
                                stop=(k == K4 - 1))
                        nc.scalar.mul(ivT_sb[:, m, :], piv[:], 1.0 / HW)

            # stage + AllGather (ind_vec^T, fa^T)
            nc.sync.dma_start(out=_rearr_kp(ag1_in[0:C, :]), in_=ivT_sb[:])
            nc.sync.dma_start(out=_rearr_kp(ag1_in[C:2 * C, :]), in_=faT_sb[:])
            nc.gpsimd.collective_compute(
                "AllGather", ALU.bypass, replica_groups=groups,
                ins=[ag1_in[:]], outs=[ag1_out[:]])
            for r in range(NCORE):
                for k in range(K4):
                    nc.sync.dma_start(
                        out=ivT_full[:, k, r * BL:(r + 1) * BL],
                        in_=ag1_out[r, k * 128:(k + 1) * 128, :])
                    nc.sync.dma_start(
                        out=faT_full[:, k, r * BL:(r + 1) * BL],
                        in_=ag1_out[r, C + k * 128:C + (k + 1) * 128, :])
            nc.sync.dma_start(out=_rearr_kp(ivt_out[:]), in_=ivT_full[:])
            nc.sync.dma_start(out=_rearr_kp(fat_out[:]), in_=faT_full[:])

            # ---------------- phase 1.5: iv_norm and U ----------------
            with tc.tile_pool(name="p15", bufs=1) as p15, \
                 tc.tile_pool(name="p15ps", bufs=2, space="PSUM") as p15ps:
                for k in range(K4):
                    nc.scalar.square(ivsq[:, k, :], ivT_full[:, k, :])
                pss = p15ps.tile([1, B], F32)
                for k in range(K4):
                    nc.tensor.matmul(out=pss[0:1, :], lhsT=ones_f[:],
                                     rhs=ivsq[:, k, :], start=(k == 0),
                                     stop=(k == K4 - 1))
                ssq = p15.tile([1, B], F32)
                nc.scalar.sqrt(ssq[:], pss[0:1, :])
                srow = p15.tile([1, B], F32)
                nc.vector.reciprocal(srow[:], ssq[:])
                t1 = p15.tile([1, B], F32)
                nc.vector.tensor_mul(t1[:], srow[:], srow[:])
                nc.vector.tensor_mul(t1[:], t1[:], pss[0:1, :])
                nc.vector.tensor_scalar(t1[:], t1[:], -0.5, 1.5,
                                        op0=ALU.mult, op1=ALU.add)
                nc.vector.tensor_mul(srow[:], srow[:], t1[:])
                nc.sync.dma_start(out=srow_dram[:], in_=srow[:])
                s_bc = p15.tile([128, B], F32)
                nc.sync.dma_start(out=s_bc[:],
                                  in_=srow_dram[:].to_broadcast([128, B]))
                for k in range(K4):
                    nc.vector.tensor_mul(ivnT_sb[:, k, :], ivT_full[:, k, :],
                                         s_bc[:])
                for m in range(K4):
                    pu = p15ps.tile([128, B], F32)
                    for k in range(K4):
                        nc.tensor.matmul(
                            out=pu[:], lhsT=Wv_sb[:, k, m * 128:(m + 1) * 128],
                            rhs=ivnT_sb[:, k, :], start=(k == 0),
                            stop=(k == K4 - 1))
                    nc.scalar.copy(UT_sb[:, m, :], pu[:])

            # ---------------- phase 2: S tiles, topk, masked sums ----------
            with tc.tile_pool(name="rbp", bufs=6) as rbpool, \
                 tc.tile_pool(name="gsp", bufs=4) as gspool, \
                 tc.tile_pool(name="sp", bufs=4) as spool, \
                 tc.tile_pool(name="zp", bufs=4) as zpool, \
                 tc.tile_pool(name="thp", bufs=2) as thpool, \
                 tc.tile_pool(name="scrp", bufs=2) as scrpool, \
                 tc.tile_pool(name="dumpp", bufs=2) as dumppool, \
                 tc.tile_pool(name="t8p", bufs=6) as t8pool, \
                 tc.tile_pool(name="gps", bufs=3, space="PSUM") as gpsum:

                def topk_and_sums(tile_sb, col, is_z):
                    """5 DVE passes for the 23rd largest, then tanh+silu."""
                    t8a = t8pool.tile([128, 8], F32, tag="t8a")
                    t8b = t8pool.tile([128, 8], F32, tag="t8b")
                    t8c = t8pool.tile([128, 8], F32, tag="t8c")
                    scr1 = scrpool.tile([128, HW], F32, tag="scr1")
                    scr2 = scrpool.tile([128, HW], F32, tag="scr2")
                    nc.vector.max(out=t8a[:], in_=tile_sb[:])
                    nc.vector.match_replace(out=scr1[:], in_to_replace=t8a[:],
                                            in_values=tile_sb[:],
                                            imm_value=BIGNEG)
                    nc.vector.max(out=t8b[:], in_=scr1[:])
                    nc.vector.match_replace(out=scr2[:], in_to_replace=t8b[:],
                                            in_values=scr1[:], imm_value=BIGNEG)
                    nc.vector.max(out=t8c[:], in_=scr2[:])
                    barr = BZ_arr if is_z else BS_arr
                    barr2 = BZ2_arr if is_z else BS2_arr
                    nc.scalar.mul(barr[:, col:col + 1],
                                  t8c[:, 6:7], -1.0 / TS2)
                    nc.scalar.mul(barr2[:, col:col + 1],
                                  t8c[:, 6:7], -1.0 / TS)
                    th = thpool.tile([128, HW], F32, tag="th")
                    t1arr = T1Z_arr if is_z else T1S_arr
                    nc.scalar.activation(th[:], tile_sb[:], AF.Tanh,
                                         bias=barr[:, col:col + 1],
                                         scale=1.0 / TS2,
                                         accum_out=t1arr[:, col:col + 1])
                    dump = dumppool.tile([128, HW], F32, tag="dump")
                    larr = LZ_arr if is_z else LS_arr
                    nc.scalar.activation(dump[:], tile_sb[:], AF.Silu,
                                         bias=barr2[:, col:col + 1],
                                         scale=1.0 / TS,
                                         accum_out=larr[:, col:col + 1])

                for pr in range(NPAIR):
                    b0, b1 = 2 * pr, 2 * pr + 1
                    evs = {}
                    rbf = {}
                    for b in (b0, b1):
                        e = evpool.tile([128, K4, HW], F32R, tag="ev")
                        nc.sync.dma_start(out=e, in_=_rearr_kp(ev[b]))
                        evs[b] = e
                        rb = rbpool.tile([128, HW], F32, tag="rb")
                        nc.sync.dma_start(
                            out=rb[:],
                            in_=recip_dram[b:b + 1, :].to_broadcast([128, HW]))
                        rbf[b] = rb
                    rbx = rbpool.tile([128, HW], F32, tag="rb")
                    nc.sync.dma_start(
                        out=rbx[0:64, :],
                        in_=recip_dram[b0:b0 + 1, :].to_broadcast([64, HW]))
                    nc.sync.dma_start(
                        out=rbx[64:128, :],
                        in_=recip_dram[b1:b1 + 1, :].to_broadcast([64, HW]))

                    # G matmuls
                    pg = {}
                    for b in (b0, b1):
                        g = gpsum.tile([128, HW], F32, tag="g")
                        for (n0, nw) in NSL:
                            for k in range(K4):
                                nc.tensor.matmul(
                                    out=g[:, n0:n0 + nw],
                                    lhsT=UT_sb[:, k, 0:128],
                                    rhs=evs[b][:, k, n0:n0 + nw],
                                    start=(k == 0), stop=(k == K4 - 1))
                        pg[b] = g
                    g1hi = {}
                    for b in (b0, b1):
                        gh = gpsum.tile([64, HW], F32, tag="g")
                        for (n0, nw) in NSL:
                            for k in range(K4):
                                nc.tensor.matmul(
                                    out=gh[:, n0:n0 + nw],
                                    lhsT=UT_sb[:, k, 128:192],
                                    rhs=evs[b][:, k, n0:n0 + nw],
                                    start=(k == 0), stop=(k == K4 - 1))
                        g1hi[b] = gh

                    # copy G to SBUF, assemble S (and running sum), Z = -S
                    tiles = []
                    for b, psrc, rbt, col in (
                            (b0, pg[b0], rbf[b0], b0),
                            (b1, pg[b1], rbf[b1], b1),
                            (None, None, rbx, BL + pr)):
                        gs = gspool.tile([128, HW], F32, tag="gs")
                        if psrc is not None:
                            nc.scalar.copy(gs[:], psrc[:])
                        else:
                            nc.scalar.copy(gs[0:64, :], g1hi[b0][:])
                            nc.scalar.copy(gs[64:128, :], g1hi[b1][:])
                        s_t = spool.tile([128, HW], F32, tag="s")
                        nc.gpsimd.tensor_mul(s_t[:], gs[:], rbt[:])
                        z_t = zpool.tile([128, HW], F32, tag="z")
                        nc.gpsimd.tensor_sub(z_t[:], zeros_hw[:], s_t[:])
                        tiles.append((s_t, z_t, col))

                    for s_t, z_t, col in tiles:
                        topk_and_sums(s_t, col, False)
                        topk_and_sums(z_t, col, True)

                # ---- batched SP/SN assembly ----
                # A = sum(m) = HW/2 + T1/2 ; sum(S*m) = TS*L + t*A
                # SP = TS*L/A + t  with  t = -TS2*BS ; SN = -(TS*Lz/Az + tz)
                AS = persist.tile([128, NCOLS], F32)
                AZ = persist.tile([128, NCOLS], F32)
                nc.vector.tensor_scalar(AS[:], T1S_arr[:], 0.5, HW / 2.0,
                                        op0=ALU.mult, op1=ALU.add)
                nc.vector.tensor_scalar(AZ[:], T1Z_arr[:], 0.5, HW / 2.0,
                                        op0=ALU.mult, op1=ALU.add)
                rAS = persist.tile([128, NCOLS], F32)
                rAZ = persist.tile([128, NCOLS], F32)
                nc.vector.reciprocal(rAS[:], AS[:])
                nc.vector.reciprocal(rAZ[:], AZ[:])
                nc.vector.tensor_mul(SP_arr[:], LS_arr[:], rAS[:])
                nc.vector.tensor_scalar(SP_arr[:], SP_arr[:], TS, None,
                                        op0=ALU.mult)
                tS = persist.tile([128, NCOLS], F32)
                nc.vector.tensor_scalar_mul(tS[:], BS_arr[:], -TS2)
                nc.vector.tensor_add(SP_arr[:], SP_arr[:], tS[:])
                nc.vector.tensor_mul(SN_arr[:], LZ_arr[:], rAZ[:])
                nc.vector.tensor_scalar(SN_arr[:], SN_arr[:], TS, None,
                                        op0=ALU.mult)
                tZ = persist.tile([128, NCOLS], F32)
                nc.vector.tensor_scalar_mul(tZ[:], BZ_arr[:], -TS2)
                nc.vector.tensor_add(SN_arr[:], SN_arr[:], tZ[:])
                nc.vector.tensor_scalar_mul(SN_arr[:], SN_arr[:], -1.0)

            # ---- stage SP^T/SN^T and AllGather ----
            nc.sync.dma_start(out=ag2_in[0, 0:128, :].rearrange("p b -> p b"),
                              in_=SP_arr[:, 0:BL])
            nc.sync.dma_start(out=ag2_in[1, 0:128, :], in_=SN_arr[:, 0:BL])
            for par in range(2):
                nc.sync.dma_start(
                    out=ag2_in[0, 128:192, par::2],
                    in_=SP_arr[par * 64:(par + 1) * 64, BL:NCOLS])
                nc.sync.dma_start(
                    out=ag2_in[1, 128:192, par::2],
                    in_=SN_arr[par * 64:(par + 1) * 64, BL:NCOLS])
            nc.gpsimd.collective_compute(
                "AllGather", ALU.bypass, replica_groups=groups,
                ins=[ag2_in[:]], outs=[ag2_out[:]])

            # ---------------- tail: losses ----------------
            if stage >= 3:
                with tc.tile_pool(name="tail", bufs=1) as tp:
                    SPT0 = tp.tile([128, B], F32)
                    SPT1 = tp.tile([64, B], F32)
                    SNT0 = tp.tile([128, B], F32)
                    SNT1 = tp.tile([64, B], F32)
                    for r in range(NCORE):
                        sl = slice(r * BL, (r + 1) * BL)
                        nc.sync.dma_start(out=SPT0[:, sl], in_=ag2_out[r, 0, 0:128, :])
                        nc.sync.dma_start(out=SPT1[:, sl], in_=ag2_out[r, 0, 128:192, :])
                        nc.sync.dma_start(out=SNT0[:, sl], in_=ag2_out[r, 1, 0:128, :])
                        nc.sync.dma_start(out=SNT1[:, sl], in_=ag2_out[r, 1, 128:192, :])
                    nc.sync.dma_start(out=spt_out[0:128, :], in_=SPT0[:])
                    nc.sync.dma_start(out=spt_out[128:192, :], in_=SPT1[:])
                    nc.sync.dma_start(out=snt_out[0:128, :], in_=SNT0[:])
                    nc.sync.dma_start(out=snt_out[128:192, :], in_=SNT1[:])

                    # transposes for the b-major (SP) view
                    SP0 = tp.tile([128, B], F32)   # b 0-127 x d 0-192
                    SP1 = tp.tile([64, B], F32)
                    SN0 = tp.tile([128, B], F32)
                    SN1 = tp.tile([64, B], F32)
                    with tc.tile_pool(name="trps", bufs=2, space="PSUM") as tps_tr:
                        for (srcs, dst0, dst1) in (((SPT0, SPT1), SP0, SP1),
                                                   ((SNT0, SNT1), SN0, SN1)):
                            s0, s1 = srcs
                            pt = tps_tr.tile([128, 128], F32, tag="tr")
                            nc.tensor.transpose(pt[:], s0[:, 0:128], ident[:])
                            nc.scalar.copy(dst0[:, 0:128], pt[:])
                            pt2 = tps_tr.tile([128, 128], F32, tag="tr")
                            nc.tensor.transpose(pt2[0:128, 0:64], s1[:, 0:128],
                                                ident[0:64, 0:64])
                            nc.scalar.copy(dst0[:, 128:192], pt2[0:128, 0:64])
                            pt3 = tps_tr.tile([128, 128], F32, tag="tr")
                            nc.tensor.transpose(pt3[0:64, 0:128], s0[:, 128:192],
                                                ident[:])
                            nc.scalar.copy(dst1[:, 0:128], pt3[0:64, 0:128])
                            pt4 = tps_tr.tile([128, 128], F32, tag="tr")
                            nc.tensor.transpose(pt4[0:64, 0:64], s1[:, 128:192],
                                                ident[0:64, 0:64])
                            nc.scalar.copy(dst1[:, 128:192], pt4[0:64, 0:64])

                    # identity slabs sized to the two chunk shapes
                    identA = tp.tile([128, B], F32)   # diag at cols 0..127
                    nc.vector.memset(identA, 0.0)
                    nc.vector.tensor_copy(identA[:, 0:128], ident[:])
                    identB = tp.tile([64, B], F32)    # rows d128.. -> diag cols 128..191
                    nc.vector.memset(identB, 0.0)
                    nc.vector.tensor_copy(identB[:, 128:192], ident[0:64, 0:64])

                    fin = tp.tile([1, 4], F32)

                    with tc.tile_pool(name="ceps", bufs=1, space="PSUM") as ceps:
                        ce_parts = ceps.tile([1, 4], F32, tag="ce")

                        def ce_sum(x0, x1, y0, y1, out_col):
                            """sum over rows of LSE([x|y]/TC) - diag(x)/TC."""
                            part_rows = []
                            for (x, idn, p) in ((x0, identA, 128), (x1, identB, 64)):
                                y = y0 if p == 128 else y1
                                m1 = tp.tile([p, 1], F32, tag=f"m1_{out_col}_{p}")
                                m2 = tp.tile([p, 1], F32, tag=f"m2_{out_col}_{p}")
                                nc.vector.tensor_reduce(out=m1[:], in_=x[:], axis=AX.X,
                                                        op=ALU.max)
                                nc.vector.tensor_reduce(out=m2[:], in_=y[:], axis=AX.X,
                                                        op=ALU.max)
                                nc.vector.tensor_tensor(out=m1[:], in0=m1[:], in1=m2[:],
                                                        op=ALU.max)
                                nbias = tp.tile([p, 1], F32, tag=f"nb_{out_col}_{p}")
                                nc.vector.tensor_scalar_mul(nbias[:], m1[:], -1.0 / TC)
                                e1 = tp.tile([p, 1], F32, tag=f"e1_{out_col}_{p}")
                                e2 = tp.tile([p, 1], F32, tag=f"e2_{out_col}_{p}")
                                dmp = tp.tile([p, B], F32, tag=f"dmp_{p}")
                                nc.scalar.activation(dmp[:], x[:], AF.Exp,
                                                     bias=nbias[:], scale=1.0 / TC,
                                                     accum_out=e1[:])
                                nc.scalar.activation(dmp[:], y[:], AF.Exp,
                                                     bias=nbias[:], scale=1.0 / TC,
                                                     accum_out=e2[:])
                                nc.vector.tensor_add(e1[:], e1[:], e2[:])
                                lse = tp.tile([p, 1], F32, tag=f"lse_{out_col}_{p}")
                                nc.scalar.activation(lse[:], e1[:], AF.Ln)
                                nc.vector.tensor_scalar(m1[:], m1[:], 1.0 / TC, None,
                                                        op0=ALU.mult)
                                nc.vector.tensor_add(lse[:], lse[:], m1[:])
                                dg = tp.tile([p, 1], F32, tag=f"dg_{out_col}_{p}")
                                dmp2 = tp.tile([p, B], F32, tag=f"dmp_{p}")
                                nc.vector.tensor_mul(dmp2[:], x[:], idn[:])
                                nc.vector.tensor_reduce(out=dg[:], in_=dmp2[:],
                                                        axis=AX.X, op=ALU.add)
                                nc.vector.tensor_scalar_mul(dg[:], dg[:],
                                                            1.0 / TC)
                                nc.vector.tensor_sub(lse[:], lse[:], dg[:])
                                part_rows.append((lse, p))
                            for i, (lse, p) in enumerate(part_rows):
                                nc.tensor.matmul(out=ce_parts[0:1, out_col:out_col + 1],
                                                 lhsT=ones_f[0:p, :], rhs=lse[:],
                                                 start=(i == 0), stop=(i == 1),
                                                 skip_group_check=True)

                        ce_sum(SP0, SP1, SN0, SN1, 0)   # loss1 * 192
                        ce_sum(SPT0, SPT1, SNT0, SNT1, 1)  # loss2 * 192
                        nc.scalar.copy(fin[:, 0:2], ce_parts[0:1, 0:2])

                    # ---- distance losses ----
                    fasq = tp.tile([128, K4, B], F32)
                    for k in range(K4):
                        nc.scalar.square(fasq[:, k, :], faT_full[:, k, :])
                    rows = tp.tile([1, 4 * B], F32)
                    with tc.tile_pool(name="rowps", bufs=1, space="PSUM") as rowps:
                        for (idx, srct) in ((0, ivsq), (1, fasq), (2, ivT_full),
                                            (3, faT_full)):
                            prow = rowps.tile([1, B], F32, tag=f"rows{idx}")
                            for k in range(K4):
                                nc.tensor.matmul(
                                    out=prow[0:1, :], lhsT=ones_f[:],
                                    rhs=srct[:, k, :], start=(k == 0),
                                    stop=(k == K4 - 1), skip_group_check=True)
                            nc.scalar.copy(rows[:, idx * B:(idx + 1) * B],
                                           prow[0:1, :])
                    niv, nfa = rows[:, 0:B], rows[:, B:2 * B]
                    siv, sfa = rows[:, 2 * B:3 * B], rows[:, 3 * B:4 * B]
                    # R_row = nfa - 2eps*sfa + C*eps^2 ; Rt_row = niv + 2eps*siv
                    Rrow = tp.tile([1, B], F32)
                    nc.vector.tensor_scalar(Rrow[:], sfa, -2.0 * EPS, C * EPS * EPS,
                                            op0=ALU.mult, op1=ALU.add)
                    nc.vector.tensor_add(Rrow[:], Rrow[:], nfa)
                    Rtrow = tp.tile([1, B], F32)
                    nc.vector.tensor_scalar(Rtrow[:], siv, 2.0 * EPS, C * EPS * EPS,
                                            op0=ALU.mult, op1=ALU.add)
                    nc.vector.tensor_add(Rtrow[:], Rtrow[:], niv)
                    nc.sync.dma_start(out=rrow_dram[:], in_=Rrow[:])
                    nc.sync.dma_start(out=rtrow_dram[:], in_=Rtrow[:])
                    Rbc = tp.tile([128, B], F32)
                    nc.sync.dma_start(out=Rbc[:], in_=rrow_dram[:].to_broadcast([128, B]))
                    Rtbc = tp.tile([128, B], F32)
                    nc.sync.dma_start(out=Rtbc[:], in_=rtrow_dram[:].to_broadcast([128, B]))
                    # column versions ([192] -> two partition chunks)
                    Ccol0 = tp.tile([128, 1], F32)
                    Ccol1 = tp.tile([64, 1], F32)
                    nc.sync.dma_start(out=Ccol0[:], in_=rtrow_dram[0, 0:128])
                    nc.sync.dma_start(out=Ccol1[:], in_=rtrow_dram[0, 128:192])
                    CcolT0 = tp.tile([128, 1], F32)
                    CcolT1 = tp.tile([64, 1], F32)
                    nc.sync.dma_start(out=CcolT0[:], in_=rrow_dram[0, 0:128])
                    nc.sync.dma_start(out=CcolT1[:], in_=rrow_dram[0, 128:192])

                    b06 = tp.tile([128, 1], F32)
                    nc.vector.memset(b06, 0.6)

                    with tc.tile_pool(name="distps", bufs=1, space="PSUM") as dps:
                        loss34_parts = dps.tile([1, 4], F32, tag="l34")

                        def dist_side(lhsTsrc, rhssrc, Rbct, Ccols, out_col):
                            """rows of relu(rowsum(dist*wm)+0.6) summed -> psum col."""
                            for ci, (p, lo) in enumerate(((128, 0), (64, 128))):
                                pcross = dps.tile([p, B], F32, tag=f"cr{ci}")
                                for k in range(K4):
                                    nc.tensor.matmul(
                                        out=pcross[:],
                                        lhsT=lhsTsrc[:, k, lo:lo + p],
                                        rhs=rhssrc[:, k, :], start=(k == 0),
                                        stop=(k == K4 - 1))
                                pvv = dps.tile([p, B], F32, tag=f"vv{ci}")
                                for k in range(K4):
                                    nc.tensor.matmul(
                                        out=pvv[:],
                                        lhsT=ivnT_sb[:, k, lo:lo + p],
                                        rhs=ivnT_sb[:, k, :], start=(k == 0),
                                        stop=(k == K4 - 1))
                                dist = tp.tile([p, B], F32, tag=f"dist{ci}")
                                nc.vector.scalar_tensor_tensor(
                                    out=dist[:], in0=pcross[:], scalar=-2.0,
                                    in1=Rbct[0:p, :], op0=ALU.mult, op1=ALU.add)
                                nc.vector.tensor_scalar_add(dist[:], dist[:],
                                                            Ccols[ci][:])
                                # wm = vv/191 off-diag, 1 on diag
                                wm = tp.tile([p, B], F32, tag=f"wm{ci}")
                                nc.vector.tensor_scalar_mul(wm[:], pvv[:],
                                                            1.0 / (B - 1))
                                idn = identA if ci == 0 else identB
                                t = tp.tile([p, B], F32, tag=f"wt{ci}")
                                nc.vector.tensor_mul(t[:], wm[:], idn[:])
                                nc.vector.tensor_sub(wm[:], wm[:], t[:])
                                nc.vector.tensor_add(wm[:], wm[:], idn[:])
                                r3 = tp.tile([p, 1], F32, tag=f"r3{ci}")
                                dmp3 = tp.tile([p, B], F32, tag=f"dmp_{p}")
                                nc.vector.tensor_mul(dmp3[:], dist[:], wm[:])
                                nc.vector.tensor_reduce(out=r3[:], in_=dmp3[:],
                                                        axis=AX.X, op=ALU.add)
                                rr = tp.tile([p, 1], F32, tag=f"rr{ci}")
                                nc.scalar.activation(rr[:], r3[:], AF.Relu,
                                                     bias=b06[0:p, :])
                                nc.tensor.matmul(
                                    out=loss34_parts[0:1, out_col:out_col + 1],
                                    lhsT=ones_f[0:p, :], rhs=rr[:], start=(ci == 0),
                                    stop=(ci == 1), skip_group_check=True)

                        dist_side(ivT_full, faT_full, Rbc, (Ccol0, Ccol1), 0)
                        dist_side(faT_full, ivT_full, Rtbc, (CcolT0, CcolT1), 1)
                        nc.scalar.copy(fin[:, 2:4], loss34_parts[0:1, 0:2])

                    # ---- final scalars ----
                    l12 = tp.tile([1, 1], F32)
                    nc.vector.tensor_add(l12[:], fin[:, 0:1], fin[:, 1:2])
                    nc.vector.tensor_scalar_mul(l12[:], l12[:], 1.0 / (2.0 * B))
                    l34 = tp.tile([1, 1], F32)
                    nc.vector.tensor_add(l34[:], fin[:, 2:3], fin[:, 3:4])
                    nc.vector.tensor_scalar_mul(l34[:], l34[:], 1.0 / (2.0 * B))
                    nc.sync.dma_start(out=loss12[:], in_=l12[:])
                    nc.sync.dma_start(out=loss34[:], in_=l34[:])

    nc.compile()
    return nc


_NC_CACHE = None


def kernel(ev, ea, Wv, Wa1, Wa2):
    global _NC_CACHE
    ev = np.ascontiguousarray(np.asarray(ev, dtype=np.float32).reshape(B, C, HW))
    ea = np.asarray(ea, dtype=np.float32)
    Wv = np.ascontiguousarray(np.asarray(Wv, dtype=np.float32))
    Wa1 = np.asarray(Wa1, dtype=np.float32)
    Wa2 = np.asarray(Wa2, dtype=np.float32)

    WvT = np.ascontiguousarray(Wv.T)
    Wa1T = np.ascontiguousarray(Wa1.T)
    Wa2T = np.ascontiguousarray(Wa2.T)

    if _NC_CACHE is None:
        _NC_CACHE = build()
    nc = _NC_CACHE

    in_maps = []
    for i in range(NCORE):
        sl = slice(i * BL, (i + 1) * BL)
        in_maps.append({
            "ev": ev[sl],
            "eaT": np.ascontiguousarray(ea[sl].T),
            "Wv": Wv,
            "WvT": WvT,
            "Wa1T": Wa1T,
            "Wa2T": Wa2T,
        })
    import os as _os
    _tr = bool(_os.environ.get("KERNEL_TRACE"))
    res = run_bass_kernel_spmd(nc, in_maps, list(range(NCORE)), trace=_tr)
    r0 = res.results[0]
    global _LAST
    _LAST = res
    l12 = np.float32(r0["loss12"][0, 0])
    l34 = np.float32(r0["loss34"][0, 0])
    return (np.asarray(l12), np.asarray(l34))


_LAST = None

